# revision 13
# baseline (speedup 1.0000x reference)
"""KGram MLP seq model (k-gram embedding lookup + 2-layer MLP + vocab projection)
on 8 Trainium2 NeuronCores.

Strategy: data-parallel over the S*B = 4096 token positions (512 rows/core,
cores 0-3 take batch 0, cores 4-7 take batch 1; each core owns a contiguous
span of 512 sequence positions of one batch column).  All weights are
replicated per core (uploaded as bf16).  Per core:

  1. indirect-DMA gather of the 640 (padded) embedding rows from E
     into a token-major [128, NG, D] tile (one DMA per 128-token group)
  2. bounce each group to a DRAM scratch [640, D] (row-major by position),
     then two big DRAM->SBUF xbar transposes into the feature-major layout
     GT[p, f, t] = E[tok[t], f*128+p]  ([128, DK, TWPAD] tile)
  3. h1^T = silu(W1^T x^T + b1) where the three K-blocks of x^T are just
     shifted column windows of GT (the k-gram windows overlap); k-outer
     loop accumulates all 8 output blocks across the 8 PSUM banks
  4. h2^T = silu(W2^T h1^T + b2)
  5. logits^T = Wout^T h2^T + bout, streamed over vocab in 1024-col groups
     (bf16 weights, f32 PSUM accumulate, bf16 output store; host upcasts)

PE warmup matmuls cover the gather/transpose window so the HAM clock never
drops; heavy DMA is split across the two HWDGE rings (sync: weights in,
scalar: logits out) with batched transfers (one DMA per weight group, one
store per 4 output tiles).

Host reassembles out[s, b, :] from the per-core logits^T shards.
"""

import math

import numpy as np
import ml_dtypes

import concourse.bass as bass
import concourse.mybir as mybir
import concourse.tile as tile
from concourse import bacc
from concourse import masks
from concourse.bass_utils import run_bass_kernel_spmd

P = 128
NCORES = 8

# Full-problem constants (hardcoded; kernel.py must be self-contained)
VOCAB = 50257
EMBED = 1024
SEQ = 2048
BATCH = 2
KGRAM = 3
VPAD = 50304  # 393 * 128
MGROUP = 1024  # vocab columns per Wout streaming group
SBATCH = 4    # output tiles per store DMA
N_WARM = 32    # head warmup chunk: cover tok DMA + first gather (PE_TP mode)
N_WARM_X = 160  # warmups covering the full gather+bounce+xbar window
PE_TP = False   # PE-mode transposes (crashes the exec unit; keep off)

_nc_cache: dict = {}


def _build(V, D, KC, T, VP, MG):
    """Build the single-core Bass graph (SPMD: same graph on all cores)."""
    DK = D // P
    TW = T + KC - 1
    NG = math.ceil(TW / P)
    TWPAD = NG * P
    NM = VP // P
    f32 = mybir.dt.float32
    bf16 = mybir.dt.bfloat16
    i32 = mybir.dt.int32
    AF = mybir.ActivationFunctionType

    nc = bacc.Bacc()

    E_d = nc.declare_dram_parameter("E", [V, D], bf16, isOutput=False)
    W1_d = nc.declare_dram_parameter("W1", [KC * D, D], bf16, isOutput=False)
    W2_d = nc.declare_dram_parameter("W2", [D, D], bf16, isOutput=False)
    Wo_d = nc.declare_dram_parameter("Wo", [D, VP], bf16, isOutput=False)
    b1_d = nc.declare_dram_parameter("b1", [P, DK], f32, isOutput=False)
    b2_d = nc.declare_dram_parameter("b2", [P, DK], f32, isOutput=False)
    bo_d = nc.declare_dram_parameter("bo", [P, NM], f32, isOutput=False)
    tok_d = nc.declare_dram_parameter("toks", [P, NG], i32, isOutput=False)
    out_d = nc.declare_dram_parameter("out", [VP, T], bf16, isOutput=True)

    with tile.TileContext(nc) as tc:
        with (
            tc.tile_pool(name="const", bufs=1) as cpool,
            tc.tile_pool(name="gath", bufs=1) as gpool,
            tc.tile_pool(name="gt", bufs=1) as gtpool,
            tc.tile_pool(name="dram", bufs=1, space="DRAM") as dpool,
            tc.tile_pool(name="w", bufs=1) as wpool,
            tc.tile_pool(name="h", bufs=1) as hpool,
            tc.tile_pool(name="wo", bufs=2) as wopool,
            tc.tile_pool(name="ot", bufs=4) as opool,
            tc.tile_pool(name="ps", bufs=6, space="PSUM") as pspool,
            tc.tile_pool(name="tp", bufs=2, space="PSUM") as tppool,
        ):
            # token indices first so the gather starts immediately
            tok_s = cpool.tile([P, NG], i32, tag="tok")
            nc.sync.dma_start(tok_s[:], tok_d[:])

            # warm tile for PE warmup matmuls; memset issues on gpsimd before
            # the gathers (which wait on the token DMA anyway)
            warm = cpool.tile([P, T], bf16, tag="warm")
            nc.gpsimd.memset(warm[:], 0.5)
            ident = cpool.tile([P, P], bf16, tag="ident")
            masks.make_identity(nc, ident[:])

            # --- embedding gather (token-major): G[p, g, :] = E[tok[p, g], :]
            G = gpool.tile([P, NG, D], bf16, tag="g", name="g")
            scratch = dpool.tile([TWPAD, D], bf16, tag="scr", name="scr")
            for g in range(NG):
                nc.gpsimd.indirect_dma_start(
                    out=G[:, g, :],
                    out_offset=None,
                    in_=E_d[:],
                    in_offset=bass.IndirectOffsetOnAxis(
                        ap=tok_s[:, g : g + 1], axis=0
                    ),
                )
                if not PE_TP:
                    # bounce group to DRAM scratch rows as soon as it lands
                    nc.scalar.dma_start(scratch[g * P : (g + 1) * P, :], G[:, g, :])

            # GT[p, f, t] = E[tok[t], f*128+p]; PE warmup matmuls burn the
            # HAM cold window while the gather/transpose pipeline runs.
            GT = gtpool.tile([P, DK, TWPAD], bf16, tag="gt", name="gt")
            warm_ps = pspool.tile([P, T], f32, tag="ps", name="warm_ps")

            def warmup(n):
                for _ in range(n):
                    nc.tensor.matmul(
                        warm_ps[:], lhsT=warm[:, :P], rhs=warm[:],
                        start=True, stop=True,
                    )

            if PE_TP:
                warmup(N_WARM)
                for g in range(NG):
                    for f in range(DK):
                        tp = tppool.tile([P, P], bf16, tag="tp", name=f"tp{g}_{f}")
                        nc.tensor.transpose(tp[:], G[:, g, f * P : (f + 1) * P], ident[:])
                        nc.any.tensor_copy(out=GT[:, f, g * P : (g + 1) * P], in_=tp[:])
                        if f == DK // 2 - 1:
                            warmup(2)  # keep the HAM busy-counter ticking
                    warmup(2)
            else:
                nc.scalar.dma_start_transpose(GT[:], scratch[:])
                warmup(N_WARM_X)

            b1_s = cpool.tile([P, DK], f32, tag="b1")
            nc.sync.dma_start(b1_s[:], b1_d[:])
            b2_s = cpool.tile([P, DK], f32, tag="b2")
            nc.sync.dma_start(b2_s[:], b2_d[:])
            bo_s = cpool.tile([P, NM], f32, tag="bo")
            nc.sync.dma_start(bo_s[:], bo_d[:])

            # --- MLP layer 1: h1^T = silu(W1^T x^T + b1) ---
            # k-outer loop: all 8 output blocks accumulate in parallel across
            # the 8 PSUM banks, so compute can start on the first GT half.
            w1_t = wpool.tile([P, KC * DK, D], bf16, tag="w1", name="w1")
            nc.sync.dma_start(w1_t[:], W1_d.rearrange("(k p) d -> p k d", p=P))
            h1 = [hpool.tile([P, T], bf16, tag=f"h1_{m}", name=f"h1_{m}") for m in range(DK)]
            for m in range(DK):
                ps = pspool.tile([P, T], f32, tag="ps")
                n = 0
                for i in range(KC):
                    for k8 in range(DK):
                        nc.tensor.matmul(
                            ps[:],
                            lhsT=w1_t[:, i * DK + k8, m * P : (m + 1) * P],
                            rhs=GT[:, k8, i : i + T],
                            start=(n == 0),
                            stop=(n == KC * DK - 1),
                        )
                        n += 1
                nc.scalar.activation(h1[m][:], ps[:], AF.Silu, bias=b1_s[:, m : m + 1])

            # --- MLP layer 2: h2^T = silu(W2^T h1^T + b2) ---
            w2_t = wpool.tile([P, DK, D], bf16, tag="w2", name="w2")
            nc.sync.dma_start(w2_t[:], W2_d.rearrange("(k p) d -> p k d", p=P))
            h2 = [hpool.tile([P, T], bf16, tag=f"h2_{m}", name=f"h2_{m}") for m in range(DK)]
            for m in range(DK):
                ps = pspool.tile([P, T], f32, tag="ps")
                for k8 in range(DK):
                    nc.tensor.matmul(
                        ps[:],
                        lhsT=w2_t[:, k8, m * P : (m + 1) * P],
                        rhs=h1[k8][:],
                        start=(k8 == 0),
                        stop=(k8 == DK - 1),
                    )
                nc.scalar.activation(h2[m][:], ps[:], AF.Silu, bias=b2_s[:, m : m + 1])

            # --- vocab projection: logits^T = Wout^T h2^T + bout ---
            Wo_v = Wo_d.rearrange("(k p) v -> p k v", p=P)
            out_v = out_d.rearrange("(q p) t -> p q t", p=P)
            c0 = 0
            while c0 < VP:
                cols = min(MG, VP - c0)
                wos = wopool.tile([P, DK, MG], bf16, tag="wo", name=f"wo{c0}")
                nc.sync.dma_start(wos[:, :, :cols], Wo_v[:, :, c0 : c0 + cols])
                nmt = cols // P
                m = 0
                while m < nmt:
                    sb = min(SBATCH, nmt - m)
                    ot = opool.tile([P, SBATCH, T], bf16, tag="ot")
                    for j in range(sb):
                        ps = pspool.tile([P, T], f32, tag="ps")
                        for k8 in range(DK):
                            nc.tensor.matmul(
                                ps[:],
                                lhsT=wos[:, k8, (m + j) * P : (m + j + 1) * P],
                                rhs=h2[k8][:],
                                start=(k8 == 0),
                                stop=(k8 == DK - 1),
                            )
                        mi = (c0 + (m + j) * P) // P
                        nc.scalar.activation(
                            ot[:, j, :], ps[:], AF.Identity, bias=bo_s[:, mi : mi + 1]
                        )
                    q0 = (c0 + m * P) // P
                    nc.scalar.dma_start(
                        out_v[:, q0 : q0 + sb, :], ot[:, :sb, :]
                    )
                    m += sb
                c0 += cols

    nc.finalize()
    return nc


def _get_nc(V, D, KC, T, VP, MG):
    key = (V, D, KC, T, VP, MG)
    if key not in _nc_cache:
        _nc_cache[key] = _build(V, D, KC, T, VP, MG)
    return _nc_cache[key]


def _run(tokens, E, W1, b1, W2, b2, Wout, bout, V, D, KC, VP, MG, trace=False):
    """tokens: (S, B) int32.  Returns (S, B, V) f32 logits (and results obj)."""
    bf16 = ml_dtypes.bfloat16
    S, B = tokens.shape
    cpb = NCORES // B  # cores per batch column
    T = S // cpb
    DK = D // P
    TW = T + KC - 1
    NG = math.ceil(TW / P)
    TWPAD = NG * P
    NM = VP // P

    E_b = E.astype(bf16)
    W1_b = W1.astype(bf16)
    W2_b = W2.astype(bf16)
    Wo_b = np.zeros((D, VP), dtype=bf16)
    Wo_b[:, :V] = Wout.astype(bf16)
    b1t = np.ascontiguousarray(b1.reshape(DK, P).T.astype(np.float32))
    b2t = np.ascontiguousarray(b2.reshape(DK, P).T.astype(np.float32))
    bo_p = np.zeros(VP, dtype=np.float32)
    bo_p[:V] = bout
    bot = np.ascontiguousarray(bo_p.reshape(NM, P).T)

    nc = _get_nc(V, D, KC, T, VP, MG)

    in_maps = []
    for c in range(NCORES):
        b, chunk = divmod(c, cpb)
        s0 = chunk * T
        pad = np.zeros(TWPAD, dtype=np.int32)
        lo = max(0, s0 - (KC - 1))
        seg = tokens[lo : s0 + T, b]
        start = (KC - 1) - (s0 - lo)
        pad[start : start + seg.size] = seg
        tok2d = np.ascontiguousarray(pad.reshape(NG, P).T)
        in_maps.append(
            {
                "E": E_b,
                "W1": W1_b,
                "W2": W2_b,
                "Wo": Wo_b,
                "b1": b1t,
                "b2": b2t,
                "bo": bot,
                "toks": tok2d,
            }
        )

    kres = run_bass_kernel_spmd(nc, in_maps, list(range(NCORES)), trace=trace)
    res = kres.results

    out = np.empty((S, B, V), dtype=np.float32)
    for c in range(NCORES):
        b, chunk = divmod(c, cpb)
        s0 = chunk * T
        out[s0 : s0 + T, b, :] = res[c]["out"][:V, :].T.astype(np.float32)
    return out, kres


def kernel(**inputs):
    tokens = np.asarray(inputs["tokens_seq"]).astype(np.int32)
    E = np.asarray(inputs["E"], dtype=np.float32)
    W1 = np.asarray(inputs["W1"], dtype=np.float32)
    b1 = np.asarray(inputs["b1"], dtype=np.float32)
    W2 = np.asarray(inputs["W2"], dtype=np.float32)
    b2 = np.asarray(inputs["b2"], dtype=np.float32)
    Wout = np.asarray(inputs["Wout"], dtype=np.float32)
    bout = np.asarray(inputs["bout"], dtype=np.float32)
    out, _ = _run(
        tokens, E, W1, b1, W2, b2, Wout, bout,
        V=VOCAB, D=EMBED, KC=KGRAM, VP=VPAD, MG=MGROUP,
    )
    return out


# revision 14
# speedup vs baseline: 1.0095x; 1.0095x over previous
"""KGram MLP seq model (k-gram embedding lookup + 2-layer MLP + vocab projection)
on 8 Trainium2 NeuronCores.

Strategy: data-parallel over the S*B = 4096 token positions (512 rows/core,
cores 0-3 take batch 0, cores 4-7 take batch 1; each core owns a contiguous
span of 512 sequence positions of one batch column).  All weights are
replicated per core (uploaded as bf16).  Per core:

  1. indirect-DMA gather of the 640 (padded) embedding rows from E
     into a token-major [128, NG, D] tile (one DMA per 128-token group)
  2. bounce each group to a DRAM scratch [640, D] (row-major by position),
     then two big DRAM->SBUF xbar transposes into the feature-major layout
     GT[p, f, t] = E[tok[t], f*128+p]  ([128, DK, TWPAD] tile)
  3. h1^T = silu(W1^T x^T + b1) where the three K-blocks of x^T are just
     shifted column windows of GT (the k-gram windows overlap); k-outer
     loop accumulates all 8 output blocks across the 8 PSUM banks
  4. h2^T = silu(W2^T h1^T + b2)
  5. logits^T = Wout^T h2^T + bout, streamed over vocab in 1024-col groups
     (bf16 weights, f32 PSUM accumulate, bf16 output store; host upcasts)

PE warmup matmuls cover the gather/transpose window so the HAM clock never
drops; heavy DMA is split across the two HWDGE rings (sync: weights in,
scalar: logits out) with batched transfers (one DMA per weight group, one
store per 4 output tiles).

Host reassembles out[s, b, :] from the per-core logits^T shards.
"""

import math

import numpy as np
import ml_dtypes

import concourse.bass as bass
import concourse.mybir as mybir
import concourse.tile as tile
from concourse import bacc
from concourse import masks
from concourse.bass_utils import run_bass_kernel_spmd

P = 128
NCORES = 8

# Full-problem constants (hardcoded; kernel.py must be self-contained)
VOCAB = 50257
EMBED = 1024
SEQ = 2048
BATCH = 2
KGRAM = 3
VPAD = 50304  # 393 * 128
MGROUP = 1024  # vocab columns per Wout streaming group
SBATCH = 4    # output tiles per store DMA
N_WARM = 32    # head warmup chunk: cover tok DMA + first gather (PE_TP mode)
N_WARM_X = 160  # warmups covering the full gather+bounce+xbar window
PE_TP = True    # PE-mode transposes

_nc_cache: dict = {}


def _build(V, D, KC, T, VP, MG):
    """Build the single-core Bass graph (SPMD: same graph on all cores)."""
    DK = D // P
    TW = T + KC - 1
    NG = math.ceil(TW / P)
    TWPAD = NG * P
    NM = VP // P
    f32 = mybir.dt.float32
    bf16 = mybir.dt.bfloat16
    i32 = mybir.dt.int32
    AF = mybir.ActivationFunctionType

    nc = bacc.Bacc()

    E_d = nc.declare_dram_parameter("E", [V, D], bf16, isOutput=False)
    W1_d = nc.declare_dram_parameter("W1", [KC * D, D], bf16, isOutput=False)
    W2_d = nc.declare_dram_parameter("W2", [D, D], bf16, isOutput=False)
    Wo_d = nc.declare_dram_parameter("Wo", [D, VP], bf16, isOutput=False)
    b1_d = nc.declare_dram_parameter("b1", [P, DK], f32, isOutput=False)
    b2_d = nc.declare_dram_parameter("b2", [P, DK], f32, isOutput=False)
    bo_d = nc.declare_dram_parameter("bo", [P, NM], f32, isOutput=False)
    tok_d = nc.declare_dram_parameter("toks", [P, NG], i32, isOutput=False)
    out_d = nc.declare_dram_parameter("out", [VP, T], bf16, isOutput=True)

    with tile.TileContext(nc) as tc:
        with (
            tc.tile_pool(name="const", bufs=1) as cpool,
            tc.tile_pool(name="gath", bufs=1) as gpool,
            tc.tile_pool(name="gt", bufs=1) as gtpool,
            tc.tile_pool(name="dram", bufs=1, space="DRAM") as dpool,
            tc.tile_pool(name="w", bufs=1) as wpool,
            tc.tile_pool(name="h", bufs=1) as hpool,
            tc.tile_pool(name="wo", bufs=2) as wopool,
            tc.tile_pool(name="ot", bufs=4) as opool,
            tc.tile_pool(name="ps", bufs=6, space="PSUM") as pspool,
            tc.tile_pool(name="tp", bufs=2, space="PSUM") as tppool,
        ):
            # token indices first so the gather starts immediately
            tok_s = cpool.tile([P, NG], i32, tag="tok")
            nc.sync.dma_start(tok_s[:], tok_d[:])

            # warm tile for PE warmup matmuls; memset issues on gpsimd before
            # the gathers (which wait on the token DMA anyway)
            warm = cpool.tile([P, T], bf16, tag="warm")
            nc.gpsimd.memset(warm[:], 0.5)
            ident = cpool.tile([P, P], bf16, tag="ident")
            masks.make_identity(nc, ident[:])

            # --- embedding gather (token-major): G[p, g, :] = E[tok[p, g], :]
            G = gpool.tile([P, NG, D], bf16, tag="g", name="g")
            scratch = dpool.tile([TWPAD, D], bf16, tag="scr", name="scr")
            for g in range(NG):
                nc.gpsimd.indirect_dma_start(
                    out=G[:, g, :],
                    out_offset=None,
                    in_=E_d[:],
                    in_offset=bass.IndirectOffsetOnAxis(
                        ap=tok_s[:, g : g + 1], axis=0
                    ),
                )
                if not PE_TP:
                    # bounce group to DRAM scratch rows as soon as it lands
                    nc.scalar.dma_start(scratch[g * P : (g + 1) * P, :], G[:, g, :])

            # GT[p, f, t] = E[tok[t], f*128+p]; PE warmup matmuls burn the
            # HAM cold window while the gather/transpose pipeline runs.
            GT = gtpool.tile([P, DK, TWPAD], bf16, tag="gt", name="gt")
            warm_ps = pspool.tile([P, T], f32, tag="ps", name="warm_ps")

            def warmup(n):
                for _ in range(n):
                    nc.tensor.matmul(
                        warm_ps[:], lhsT=warm[:, :P], rhs=warm[:],
                        start=True, stop=True,
                    )

            if PE_TP:
                warmup(N_WARM)
                for g in range(NG):
                    for f in range(DK):
                        tp = tppool.tile([P, P], bf16, tag="tp", name=f"tp{g}_{f}")
                        nc.tensor.transpose(tp[:], G[:, g, f * P : (f + 1) * P], ident[:])
                        nc.scalar.activation(
                            GT[:, f, g * P : (g + 1) * P], tp[:], AF.Identity
                        )
            else:
                nc.scalar.dma_start_transpose(GT[:], scratch[:])
                warmup(N_WARM_X)

            b1_s = cpool.tile([P, DK], f32, tag="b1")
            nc.sync.dma_start(b1_s[:], b1_d[:])
            b2_s = cpool.tile([P, DK], f32, tag="b2")
            nc.sync.dma_start(b2_s[:], b2_d[:])
            bo_s = cpool.tile([P, NM], f32, tag="bo")
            nc.sync.dma_start(bo_s[:], bo_d[:])

            # --- MLP layer 1: h1^T = silu(W1^T x^T + b1) ---
            # k-outer loop: all 8 output blocks accumulate in parallel across
            # the 8 PSUM banks, so compute can start on the first GT half.
            w1_t = wpool.tile([P, KC * DK, D], bf16, tag="w1", name="w1")
            nc.sync.dma_start(w1_t[:], W1_d.rearrange("(k p) d -> p k d", p=P))
            h1 = [hpool.tile([P, T], bf16, tag=f"h1_{m}", name=f"h1_{m}") for m in range(DK)]
            for m in range(DK):
                ps = pspool.tile([P, T], f32, tag="ps")
                n = 0
                for i in range(KC):
                    for k8 in range(DK):
                        nc.tensor.matmul(
                            ps[:],
                            lhsT=w1_t[:, i * DK + k8, m * P : (m + 1) * P],
                            rhs=GT[:, k8, i : i + T],
                            start=(n == 0),
                            stop=(n == KC * DK - 1),
                        )
                        n += 1
                nc.scalar.activation(h1[m][:], ps[:], AF.Silu, bias=b1_s[:, m : m + 1])

            # --- MLP layer 2: h2^T = silu(W2^T h1^T + b2) ---
            w2_t = wpool.tile([P, DK, D], bf16, tag="w2", name="w2")
            nc.sync.dma_start(w2_t[:], W2_d.rearrange("(k p) d -> p k d", p=P))
            h2 = [hpool.tile([P, T], bf16, tag=f"h2_{m}", name=f"h2_{m}") for m in range(DK)]
            for m in range(DK):
                ps = pspool.tile([P, T], f32, tag="ps")
                for k8 in range(DK):
                    nc.tensor.matmul(
                        ps[:],
                        lhsT=w2_t[:, k8, m * P : (m + 1) * P],
                        rhs=h1[k8][:],
                        start=(k8 == 0),
                        stop=(k8 == DK - 1),
                    )
                nc.scalar.activation(h2[m][:], ps[:], AF.Silu, bias=b2_s[:, m : m + 1])

            # --- vocab projection: logits^T = Wout^T h2^T + bout ---
            Wo_v = Wo_d.rearrange("(k p) v -> p k v", p=P)
            out_v = out_d.rearrange("(q p) t -> p q t", p=P)
            c0 = 0
            while c0 < VP:
                cols = min(MG, VP - c0)
                wos = wopool.tile([P, DK, MG], bf16, tag="wo", name=f"wo{c0}")
                nc.sync.dma_start(wos[:, :, :cols], Wo_v[:, :, c0 : c0 + cols])
                nmt = cols // P
                m = 0
                while m < nmt:
                    sb = min(SBATCH, nmt - m)
                    ot = opool.tile([P, SBATCH, T], bf16, tag="ot")
                    for j in range(sb):
                        ps = pspool.tile([P, T], f32, tag="ps")
                        for k8 in range(DK):
                            nc.tensor.matmul(
                                ps[:],
                                lhsT=wos[:, k8, (m + j) * P : (m + j + 1) * P],
                                rhs=h2[k8][:],
                                start=(k8 == 0),
                                stop=(k8 == DK - 1),
                            )
                        mi = (c0 + (m + j) * P) // P
                        nc.scalar.activation(
                            ot[:, j, :], ps[:], AF.Identity, bias=bo_s[:, mi : mi + 1]
                        )
                    q0 = (c0 + m * P) // P
                    nc.scalar.dma_start(
                        out_v[:, q0 : q0 + sb, :], ot[:, :sb, :]
                    )
                    m += sb
                c0 += cols

    nc.finalize()
    return nc


def _get_nc(V, D, KC, T, VP, MG):
    key = (V, D, KC, T, VP, MG)
    if key not in _nc_cache:
        _nc_cache[key] = _build(V, D, KC, T, VP, MG)
    return _nc_cache[key]


def _run(tokens, E, W1, b1, W2, b2, Wout, bout, V, D, KC, VP, MG, trace=False):
    """tokens: (S, B) int32.  Returns (S, B, V) f32 logits (and results obj)."""
    bf16 = ml_dtypes.bfloat16
    S, B = tokens.shape
    cpb = NCORES // B  # cores per batch column
    T = S // cpb
    DK = D // P
    TW = T + KC - 1
    NG = math.ceil(TW / P)
    TWPAD = NG * P
    NM = VP // P

    E_b = E.astype(bf16)
    W1_b = W1.astype(bf16)
    W2_b = W2.astype(bf16)
    Wo_b = np.zeros((D, VP), dtype=bf16)
    Wo_b[:, :V] = Wout.astype(bf16)
    b1t = np.ascontiguousarray(b1.reshape(DK, P).T.astype(np.float32))
    b2t = np.ascontiguousarray(b2.reshape(DK, P).T.astype(np.float32))
    bo_p = np.zeros(VP, dtype=np.float32)
    bo_p[:V] = bout
    bot = np.ascontiguousarray(bo_p.reshape(NM, P).T)

    nc = _get_nc(V, D, KC, T, VP, MG)

    in_maps = []
    for c in range(NCORES):
        b, chunk = divmod(c, cpb)
        s0 = chunk * T
        pad = np.zeros(TWPAD, dtype=np.int32)
        lo = max(0, s0 - (KC - 1))
        seg = tokens[lo : s0 + T, b]
        start = (KC - 1) - (s0 - lo)
        pad[start : start + seg.size] = seg
        tok2d = np.ascontiguousarray(pad.reshape(NG, P).T)
        in_maps.append(
            {
                "E": E_b,
                "W1": W1_b,
                "W2": W2_b,
                "Wo": Wo_b,
                "b1": b1t,
                "b2": b2t,
                "bo": bot,
                "toks": tok2d,
            }
        )

    kres = run_bass_kernel_spmd(nc, in_maps, list(range(NCORES)), trace=trace)
    res = kres.results

    out = np.empty((S, B, V), dtype=np.float32)
    for c in range(NCORES):
        b, chunk = divmod(c, cpb)
        s0 = chunk * T
        out[s0 : s0 + T, b, :] = res[c]["out"][:V, :].T.astype(np.float32)
    return out, kres


def kernel(**inputs):
    tokens = np.asarray(inputs["tokens_seq"]).astype(np.int32)
    E = np.asarray(inputs["E"], dtype=np.float32)
    W1 = np.asarray(inputs["W1"], dtype=np.float32)
    b1 = np.asarray(inputs["b1"], dtype=np.float32)
    W2 = np.asarray(inputs["W2"], dtype=np.float32)
    b2 = np.asarray(inputs["b2"], dtype=np.float32)
    Wout = np.asarray(inputs["Wout"], dtype=np.float32)
    bout = np.asarray(inputs["bout"], dtype=np.float32)
    out, _ = _run(
        tokens, E, W1, b1, W2, b2, Wout, bout,
        V=VOCAB, D=EMBED, KC=KGRAM, VP=VPAD, MG=MGROUP,
    )
    return out


# revision 15
# speedup vs baseline: 1.0107x; 1.0012x over previous
"""KGram MLP seq model (k-gram embedding lookup + 2-layer MLP + vocab projection)
on 8 Trainium2 NeuronCores.

Strategy: data-parallel over the S*B = 4096 token positions (512 rows/core,
cores 0-3 take batch 0, cores 4-7 take batch 1; each core owns a contiguous
span of 512 sequence positions of one batch column).  All weights are
replicated per core (uploaded as bf16).  Per core:

  1. indirect-DMA gather of the 640 (padded) embedding rows from E
     into a token-major [128, NG, D] tile (one DMA per 128-token group)
  2. bounce each group to a DRAM scratch [640, D] (row-major by position),
     then two big DRAM->SBUF xbar transposes into the feature-major layout
     GT[p, f, t] = E[tok[t], f*128+p]  ([128, DK, TWPAD] tile)
  3. h1^T = silu(W1^T x^T + b1) where the three K-blocks of x^T are just
     shifted column windows of GT (the k-gram windows overlap); k-outer
     loop accumulates all 8 output blocks across the 8 PSUM banks
  4. h2^T = silu(W2^T h1^T + b2)
  5. logits^T = Wout^T h2^T + bout, streamed over vocab in 1024-col groups
     (bf16 weights, f32 PSUM accumulate, bf16 output store; host upcasts)

PE warmup matmuls cover the gather/transpose window so the HAM clock never
drops; heavy DMA is split across the two HWDGE rings (sync: weights in,
scalar: logits out) with batched transfers (one DMA per weight group, one
store per 4 output tiles).

Host reassembles out[s, b, :] from the per-core logits^T shards.
"""

import math

import numpy as np
import ml_dtypes

import concourse.bass as bass
import concourse.mybir as mybir
import concourse.tile as tile
from concourse import bacc
from concourse import masks
from concourse.bass_utils import run_bass_kernel_spmd

P = 128
NCORES = 8

# Full-problem constants (hardcoded; kernel.py must be self-contained)
VOCAB = 50257
EMBED = 1024
SEQ = 2048
BATCH = 2
KGRAM = 3
VPAD = 50304  # 393 * 128
MGROUP = 1024  # vocab columns per Wout streaming group
SBATCH = 4    # output tiles per store DMA
N_WARM = 32    # head warmup chunk: cover tok DMA + first gather (PE_TP mode)
N_WARM_X = 160  # warmups covering the full gather+bounce+xbar window
PE_TP = True    # PE-mode transposes

_nc_cache: dict = {}


def _build(V, D, KC, T, VP, MG):
    """Build the single-core Bass graph (SPMD: same graph on all cores)."""
    DK = D // P
    TW = T + KC - 1
    NG = math.ceil(TW / P)
    TWPAD = NG * P
    NM = VP // P
    f32 = mybir.dt.float32
    bf16 = mybir.dt.bfloat16
    i32 = mybir.dt.int32
    AF = mybir.ActivationFunctionType

    nc = bacc.Bacc()

    E_d = nc.declare_dram_parameter("E", [V, D], bf16, isOutput=False)
    W1_d = nc.declare_dram_parameter("W1", [KC * D, D], bf16, isOutput=False)
    W2_d = nc.declare_dram_parameter("W2", [D, D], bf16, isOutput=False)
    Wo_d = nc.declare_dram_parameter("Wo", [D, VP], bf16, isOutput=False)
    b1_d = nc.declare_dram_parameter("b1", [P, DK], f32, isOutput=False)
    b2_d = nc.declare_dram_parameter("b2", [P, DK], f32, isOutput=False)
    bo_d = nc.declare_dram_parameter("bo", [P, NM], f32, isOutput=False)
    tok_d = nc.declare_dram_parameter("toks", [P, NG], i32, isOutput=False)
    out_d = nc.declare_dram_parameter("out", [VP, T], bf16, isOutput=True)

    with tile.TileContext(nc) as tc:
        with (
            tc.tile_pool(name="const", bufs=1) as cpool,
            tc.tile_pool(name="gath", bufs=1) as gpool,
            tc.tile_pool(name="gt", bufs=1) as gtpool,
            tc.tile_pool(name="dram", bufs=1, space="DRAM") as dpool,
            tc.tile_pool(name="w", bufs=1) as wpool,
            tc.tile_pool(name="h", bufs=1) as hpool,
            tc.tile_pool(name="wo", bufs=2) as wopool,
            tc.tile_pool(name="ot", bufs=4) as opool,
            tc.tile_pool(name="ps", bufs=6, space="PSUM") as pspool,
            tc.tile_pool(name="tp", bufs=2, space="PSUM") as tppool,
        ):
            # token indices first so the gather starts immediately
            tok_s = cpool.tile([P, NG], i32, tag="tok")
            nc.sync.dma_start(tok_s[:], tok_d[:])

            # warm tile for PE warmup matmuls; memset issues on gpsimd before
            # the gathers (which wait on the token DMA anyway)
            warm = cpool.tile([P, T], bf16, tag="warm")
            nc.gpsimd.memset(warm[:], 0.5)
            ident = cpool.tile([P, P], bf16, tag="ident")
            masks.make_identity(nc, ident[:])

            # --- embedding gather (token-major): G[p, g, :] = E[tok[p, g], :]
            G = gpool.tile([P, NG, D], bf16, tag="g", name="g")
            scratch = dpool.tile([TWPAD, D], bf16, tag="scr", name="scr")
            for g in range(NG):
                nc.gpsimd.indirect_dma_start(
                    out=G[:, g, :],
                    out_offset=None,
                    in_=E_d[:],
                    in_offset=bass.IndirectOffsetOnAxis(
                        ap=tok_s[:, g : g + 1], axis=0
                    ),
                )
                if not PE_TP:
                    # bounce group to DRAM scratch rows as soon as it lands
                    nc.scalar.dma_start(scratch[g * P : (g + 1) * P, :], G[:, g, :])

            # GT[p, f, t] = E[tok[t], f*128+p]; PE warmup matmuls burn the
            # HAM cold window while the gather/transpose pipeline runs.
            GT = gtpool.tile([P, DK, TWPAD], bf16, tag="gt", name="gt")
            warm_ps = pspool.tile([P, T], f32, tag="ps", name="warm_ps")

            def warmup(n):
                for _ in range(n):
                    nc.tensor.matmul(
                        warm_ps[:], lhsT=warm[:, :P], rhs=warm[:],
                        start=True, stop=True,
                    )

            if PE_TP:
                warmup(N_WARM)
                for g in range(NG):
                    for f in range(DK):
                        tp = tppool.tile([P, P], bf16, tag="tp", name=f"tp{g}_{f}")
                        nc.tensor.transpose(tp[:], G[:, g, f * P : (f + 1) * P], ident[:])
                        nc.scalar.activation(
                            GT[:, f, g * P : (g + 1) * P], tp[:], AF.Identity
                        )
                        if f == DK // 2 - 1:
                            warmup(2)  # keep the HAM busy-counter ticking
                    warmup(2)
            else:
                nc.scalar.dma_start_transpose(GT[:], scratch[:])
                warmup(N_WARM_X)

            b1_s = cpool.tile([P, DK], f32, tag="b1")
            nc.sync.dma_start(b1_s[:], b1_d[:])
            b2_s = cpool.tile([P, DK], f32, tag="b2")
            nc.sync.dma_start(b2_s[:], b2_d[:])
            bo_s = cpool.tile([P, NM], f32, tag="bo")
            nc.sync.dma_start(bo_s[:], bo_d[:])

            # --- MLP layer 1: h1^T = silu(W1^T x^T + b1) ---
            # k-outer loop: all 8 output blocks accumulate in parallel across
            # the 8 PSUM banks, so compute can start on the first GT half.
            w1_t = wpool.tile([P, KC * DK, D], bf16, tag="w1", name="w1")
            nc.sync.dma_start(w1_t[:], W1_d.rearrange("(k p) d -> p k d", p=P))
            h1 = [hpool.tile([P, T], bf16, tag=f"h1_{m}", name=f"h1_{m}") for m in range(DK)]
            for m in range(DK):
                ps = pspool.tile([P, T], f32, tag="ps")
                n = 0
                for i in range(KC):
                    for k8 in range(DK):
                        nc.tensor.matmul(
                            ps[:],
                            lhsT=w1_t[:, i * DK + k8, m * P : (m + 1) * P],
                            rhs=GT[:, k8, i : i + T],
                            start=(n == 0),
                            stop=(n == KC * DK - 1),
                        )
                        n += 1
                nc.scalar.activation(h1[m][:], ps[:], AF.Silu, bias=b1_s[:, m : m + 1])

            # --- MLP layer 2: h2^T = silu(W2^T h1^T + b2) ---
            w2_t = wpool.tile([P, DK, D], bf16, tag="w2", name="w2")
            nc.sync.dma_start(w2_t[:], W2_d.rearrange("(k p) d -> p k d", p=P))
            h2 = [hpool.tile([P, T], bf16, tag=f"h2_{m}", name=f"h2_{m}") for m in range(DK)]
            for m in range(DK):
                ps = pspool.tile([P, T], f32, tag="ps")
                for k8 in range(DK):
                    nc.tensor.matmul(
                        ps[:],
                        lhsT=w2_t[:, k8, m * P : (m + 1) * P],
                        rhs=h1[k8][:],
                        start=(k8 == 0),
                        stop=(k8 == DK - 1),
                    )
                nc.scalar.activation(h2[m][:], ps[:], AF.Silu, bias=b2_s[:, m : m + 1])

            # --- vocab projection: logits^T = Wout^T h2^T + bout ---
            Wo_v = Wo_d.rearrange("(k p) v -> p k v", p=P)
            out_v = out_d.rearrange("(q p) t -> p q t", p=P)
            c0 = 0
            while c0 < VP:
                cols = min(MG, VP - c0)
                wos = wopool.tile([P, DK, MG], bf16, tag="wo", name=f"wo{c0}")
                nc.sync.dma_start(wos[:, :, :cols], Wo_v[:, :, c0 : c0 + cols])
                nmt = cols // P
                m = 0
                while m < nmt:
                    sb = min(SBATCH, nmt - m)
                    ot = opool.tile([P, SBATCH, T], bf16, tag="ot")
                    for j in range(sb):
                        ps = pspool.tile([P, T], f32, tag="ps")
                        for k8 in range(DK):
                            nc.tensor.matmul(
                                ps[:],
                                lhsT=wos[:, k8, (m + j) * P : (m + j + 1) * P],
                                rhs=h2[k8][:],
                                start=(k8 == 0),
                                stop=(k8 == DK - 1),
                            )
                        mi = (c0 + (m + j) * P) // P
                        nc.scalar.activation(
                            ot[:, j, :], ps[:], AF.Identity, bias=bo_s[:, mi : mi + 1]
                        )
                    q0 = (c0 + m * P) // P
                    nc.scalar.dma_start(
                        out_v[:, q0 : q0 + sb, :], ot[:, :sb, :]
                    )
                    m += sb
                c0 += cols

    nc.finalize()
    return nc


def _get_nc(V, D, KC, T, VP, MG):
    key = (V, D, KC, T, VP, MG)
    if key not in _nc_cache:
        _nc_cache[key] = _build(V, D, KC, T, VP, MG)
    return _nc_cache[key]


def _run(tokens, E, W1, b1, W2, b2, Wout, bout, V, D, KC, VP, MG, trace=False):
    """tokens: (S, B) int32.  Returns (S, B, V) f32 logits (and results obj)."""
    bf16 = ml_dtypes.bfloat16
    S, B = tokens.shape
    cpb = NCORES // B  # cores per batch column
    T = S // cpb
    DK = D // P
    TW = T + KC - 1
    NG = math.ceil(TW / P)
    TWPAD = NG * P
    NM = VP // P

    E_b = E.astype(bf16)
    W1_b = W1.astype(bf16)
    W2_b = W2.astype(bf16)
    Wo_b = np.zeros((D, VP), dtype=bf16)
    Wo_b[:, :V] = Wout.astype(bf16)
    b1t = np.ascontiguousarray(b1.reshape(DK, P).T.astype(np.float32))
    b2t = np.ascontiguousarray(b2.reshape(DK, P).T.astype(np.float32))
    bo_p = np.zeros(VP, dtype=np.float32)
    bo_p[:V] = bout
    bot = np.ascontiguousarray(bo_p.reshape(NM, P).T)

    nc = _get_nc(V, D, KC, T, VP, MG)

    in_maps = []
    for c in range(NCORES):
        b, chunk = divmod(c, cpb)
        s0 = chunk * T
        pad = np.zeros(TWPAD, dtype=np.int32)
        lo = max(0, s0 - (KC - 1))
        seg = tokens[lo : s0 + T, b]
        start = (KC - 1) - (s0 - lo)
        pad[start : start + seg.size] = seg
        tok2d = np.ascontiguousarray(pad.reshape(NG, P).T)
        in_maps.append(
            {
                "E": E_b,
                "W1": W1_b,
                "W2": W2_b,
                "Wo": Wo_b,
                "b1": b1t,
                "b2": b2t,
                "bo": bot,
                "toks": tok2d,
            }
        )

    kres = run_bass_kernel_spmd(nc, in_maps, list(range(NCORES)), trace=trace)
    res = kres.results

    out = np.empty((S, B, V), dtype=np.float32)
    for c in range(NCORES):
        b, chunk = divmod(c, cpb)
        s0 = chunk * T
        out[s0 : s0 + T, b, :] = res[c]["out"][:V, :].T.astype(np.float32)
    return out, kres


def kernel(**inputs):
    tokens = np.asarray(inputs["tokens_seq"]).astype(np.int32)
    E = np.asarray(inputs["E"], dtype=np.float32)
    W1 = np.asarray(inputs["W1"], dtype=np.float32)
    b1 = np.asarray(inputs["b1"], dtype=np.float32)
    W2 = np.asarray(inputs["W2"], dtype=np.float32)
    b2 = np.asarray(inputs["b2"], dtype=np.float32)
    Wout = np.asarray(inputs["Wout"], dtype=np.float32)
    bout = np.asarray(inputs["bout"], dtype=np.float32)
    out, _ = _run(
        tokens, E, W1, b1, W2, b2, Wout, bout,
        V=VOCAB, D=EMBED, KC=KGRAM, VP=VPAD, MG=MGROUP,
    )
    return out
